# revision 14
# baseline (speedup 1.0000x reference)
"""Trainium2 Bass kernel for nn_Attention_9122510537215 (gnn_message_passing).

Math (per batch b):
    Q = query @ Wq.T + bq                  [LQ=256, 256]
    K = input @ Wk.T + bk                  [LK, 256]
    V = input @ Wv.T + bv                  [LK, 256]
    alpha = softmax_k(Q @ K.T / 16)        [256, LK]
    out[j] = sum_k alpha[j, k] * V[k, j]   [256]

Algebraic restructuring:
  * bk shifts every score column by a constant along k -> cancels in softmax_k.
  * G[b] = Wk.T @ (query_b @ Wq.T + bq).T / 16, so scoresT = input @ G  ([LK, 256]).
  * Instead of materializing V, accumulate H[j, i] = sum_k e[k, j] * input[k, i]
    (e = exp(scores)); numer[j] = sum_i H[j, i] * Wv[j, i]; an appended
    ones-column yields denom[j] = H[j, 256]; out = numer / denom + bv.
  * Softmax is computed unnormalized without max-subtraction (scores are O(1)).

Performance structure (vs the fp16 predecessor):
  * All matmuls run in fp8e4 (e4m3) with MatmulPerfMode.DoubleRow: two
    128-row contraction tiles per pass at 0.5 cycles/output-column.
      - scores: per 128-node subtile, ONE DR matmul contracts all 256 input
        features (xt laid out [i_lo(128 part), 2(i_hi), k]).
      - H: subtile PAIRS contract 256 nodes per pass (e laid out
        [k(128 part), 2(pair), j]; xn natural [k, 2(pair), 257]).
  * exp is the serial bottleneck (B*LQ*LK/8 = 6.4M exps/core), so it is
    SPLIT across two engines: ScalarE computes exact Exp (fp8 out,
    scale=1/SG), and VectorE computes a Schraudolph-style exp by writing
    round(A8*score + B8) as int8 and BITCASTING those bytes as fp8e4
    (weights' 3.3% rms wiggle averages out over 50k softmax terms).
  * Whole per-core input (12.9 MB fp8) is SBUF-resident; chunked DMAs on a
    single ordered sync queue overlap the whole compute pipeline.
  * Distribution: LK padded to 50176 = 8*6272, sharded over 8 cores; each
    core returns fp16 partial H [B, 2, 128, 257]; host reduces in float64.
"""

import numpy as np
from contextlib import ExitStack

import concourse.bass as bass
import concourse.mybir as mybir
import concourse.tile as tile
from concourse import bacc
from concourse.bass_utils import run_bass_kernel_spmd

# Problem constants (hardcoded; kernel.py must be self-contained).
B = 4
LQ = 256
LK = 50000
OUT = 256
KV = 256            # input feature dim
NORM = 1.0 / 16.0   # 1/sqrt(OUT)

N_CORES = 8
SUB = 128                  # nodes per subtile (PE contraction width)
NSUB = 49                  # subtiles per core per batch
KS = NSUB * SUB            # 6272 nodes per core per batch
LK_PAD = KS * N_CORES      # 50176
GRP = 2                    # subtiles per exp/psum group (1 DoubleRow pair,
NGRP = NSUB // GRP         # 1 PSUM bank) -> 24 groups; subtile 48 is odd
CHUNKS = (4, 10, 12, 12, 11)  # subtiles per DMA chunk: small head chunk
NCHUNK = len(CHUNKS)          # starts compute early; boundaries stay even
CH_OFF = tuple(sum(CHUNKS[:i]) for i in range(NCHUNK))

SG = 64.0                        # score scale inside PSUM (folded into g)
A8 = 8 * np.log2(np.e) / SG      # Schraudolph fp8e4 slope
B8 = 56.05                       # 8*7 bias, +0.5 trunc->round, -0.45 mean-cal

# 14 ScalarE groups / 10 VectorE groups (+ odd tail on VectorE) per batch,
# alternating so the two exp engines run concurrently.
ACT_GROUPS = frozenset(
    tuple(i for i in range(24) if i % 2 == 0) + (5, 17))
PEND = 3                   # H matmuls lag their group by 3 so the PE FIFO
                           # never blocks upcoming scores on a pending exp

F8 = mybir.dt.float8e4
F16 = mybir.dt.float16
F32 = mybir.dt.float32
I8 = mybir.dt.int8
NPF8 = mybir.dt.np(mybir.dt.float8e4)
DR = mybir.MatmulPerfMode.DoubleRow


def _chunk_of(s):
    """DMA chunk index and subtile offset within the chunk."""
    for c in range(NCHUNK - 1, -1, -1):
        if s >= CH_OFF[c]:
            return c, s - CH_OFF[c]
    raise AssertionError(s)


def build():
    """Emit the per-core SPMD Bass module (identical on all cores)."""
    nc = bacc.Bacc("TRN2", target_bir_lowering=False, debug=False,
                   num_devices=N_CORES)
    xn = nc.dram_tensor("xn", [B, 128, NSUB, 258], F8, kind="ExternalInput")
    xt = nc.dram_tensor("xt", [B, 128, 2, KS], F8, kind="ExternalInput")
    # g[..., 0, :] = fp8(G), g[..., 1, :] = fp8(G - fp8(G)): a second
    # accumulating matmul restores ~11-bit G precision (the output error is
    # dominated by the NON-averaging diagonal term delta_G[q,q]*Var(x)).
    g = nc.dram_tensor("g", [128, B, 2, 2, 256], F8, kind="ExternalInput")
    ht = nc.dram_tensor("ht", [B, 2, 128, 257], F16, kind="ExternalOutput")

    with ExitStack() as ctx:
        tc = ctx.enter_context(tile.TileContext(nc))
        gp = ctx.enter_context(tc.tile_pool(name="gp", bufs=1))
        xnp = ctx.enter_context(tc.tile_pool(name="xnp", bufs=B * NCHUNK))
        xtp = ctx.enter_context(tc.tile_pool(name="xtp", bufs=B * NCHUNK))
        ep = ctx.enter_context(tc.tile_pool(name="ep", bufs=4))
        eip = ctx.enter_context(tc.tile_pool(name="eip", bufs=4))
        hop = ctx.enter_context(tc.tile_pool(name="hop", bufs=2))
        spp = ctx.enter_context(tc.tile_pool(name="spp", bufs=6, space="PSUM"))
        hpp = ctx.enter_context(tc.tile_pool(name="hpp", bufs=1, space="PSUM"))

        # G hi+lo for all batches, resident: [i_lo(128 part), b, i_hi, hl, q].
        g_sb = gp.tile([128, B, 2, 2, 256], F8)
        nc.sync.dma_start(out=g_sb[:, :, :, :, :], in_=g[:, :, :, :, :])

        # Chunked input loads on two queues (xt via sync/SP HWDGE, xn via
        # gpsimd/Pool SWDGE), interleaved so batch 0's first chunks land
        # first; scores only need xt+g, so they start before xn arrives.
        xn_tiles = {}
        xt_tiles = {}
        cmax = max(CHUNKS)
        for b in range(B):
            for c in range(NCHUNK):
                s0, ns = CH_OFF[c], CHUNKS[c]
                xtt = xtp.tile([128, 2, cmax * SUB], F8, tag="xt")
                nc.sync.dma_start(
                    out=xtt[:, :, :ns * SUB],
                    in_=xt[b, :, :, s0 * SUB:(s0 + ns) * SUB])
                xt_tiles[(b, c)] = xtt
                xnt = xnp.tile([128, cmax, 258], F8, tag="xn")
                nc.gpsimd.dma_start(out=xnt[:, :ns, :],
                                    in_=xn[b, :, s0:s0 + ns, :])
                xn_tiles[(b, c)] = xnt

        # groups: (subtile list, is_act) per batch; odd tail rides VectorE.
        groups = [([g_ * GRP + i for i in range(GRP)], g_ in ACT_GROUPS)
                  for g_ in range(NGRP)]
        groups.append(([NSUB - 1], False))

        for b in range(B):
            ht0 = hpp.tile([128, 257], F32, tag="ht0")
            ht1 = hpp.tile([128, 257], F32, tag="ht1")
            pend = []     # groups whose H matmuls haven't been issued yet
            popped = 0
            started = False

            for item in groups + [None] * PEND:
                if item is not None:
                    subs, is_act = item
                    sz = len(subs)
                    sp = spp.tile([128, GRP, 256], F32)
                    for i, s in enumerate(subs):
                        c, off = _chunk_of(s)
                        xtt = xt_tiles[(b, c)]
                        for hl in range(2):  # G hi then lo residual
                            nc.tensor.matmul(
                                sp[:, i, :],
                                xtt[:, :, off * SUB:(off + 1) * SUB],
                                g_sb[:, b, :, hl, :],
                                start=hl == 0, stop=hl == 1, perf_mode=DR)
                    if is_act:
                        e = ep.tile([128, GRP, 256], F8, tag="ea")
                        nc.scalar.activation(
                            e[:, :sz, :], sp[:, :sz, :],
                            mybir.ActivationFunctionType.Exp, scale=1.0 / SG)
                        e_f8 = e
                    else:
                        ei = eip.tile([128, GRP, 256], I8, tag="ei")
                        nc.vector.tensor_scalar(
                            ei[:, :sz, :], sp[:, :sz, :], A8, B8,
                            mybir.AluOpType.mult, mybir.AluOpType.add)
                        e_f8 = ei[:, :, :].bitcast(F8)
                    pend.append((e_f8, subs))
                # H matmuls lag PEND groups behind so the PE FIFO never
                # stalls the next group's scores on an exp result.
                if len(pend) > PEND or (item is None and pend):
                    pe_, psubs = pend.pop(0)
                    popped += 1
                    is_last_grp = popped == len(groups)
                    npair = len(psubs) // 2
                    for i in range(npair):
                        s = psubs[2 * i]
                        c, off = _chunk_of(s)
                        xnt = xn_tiles[(b, c)]
                        first = not started
                        started = True
                        last = (is_last_grp and i == npair - 1
                                and len(psubs) % 2 == 0)
                        for h, hacc in ((0, ht0), (1, ht1)):
                            nc.tensor.matmul(
                                hacc[:, :],
                                pe_[:, 2 * i:2 * i + 2, h * 128:(h + 1) * 128],
                                xnt[:, off:off + 2, 0:257],
                                start=first, stop=last, perf_mode=DR)
                    if len(psubs) % 2:  # odd tail: plain fp8 matmul
                        s = psubs[-1]
                        c, off = _chunk_of(s)
                        xnt = xn_tiles[(b, c)]
                        i = len(psubs) - 1
                        first = not started
                        started = True
                        for h, hacc in ((0, ht0), (1, ht1)):
                            nc.tensor.matmul(
                                hacc[:, :],
                                pe_[:, i, h * 128:(h + 1) * 128],
                                xnt[:, off, 0:257],
                                start=first, stop=is_last_grp)

            hts = hop.tile([128, 2, 257], F16)
            nc.vector.tensor_copy(hts[:, 0, :], ht0[:, :])
            nc.vector.tensor_copy(hts[:, 1, :], ht1[:, :])
            nc.gpsimd.dma_start(out=ht[b, 0], in_=hts[:, 0, :])
            nc.gpsimd.dma_start(out=ht[b, 1], in_=hts[:, 1, :])
    nc.compile()
    return nc


def _prepare_inputs(query, input, Wq, bq, Wk):
    """Host marshalling: folded G + fp8 input in both layouts, k-sharded."""
    # G[b] = Wk.T @ (query_b @ Wq.T + bq).T -> [B, 256(i), 256(q)], then
    # * NORM (1/16) * SG (64) so PSUM scores arrive pre-scaled by SG.
    Q = query.astype(np.float64) @ Wq.T.astype(np.float64) + bq
    G = np.einsum('di,bqd->biq', Wk.astype(np.float64), Q) * (NORM * SG)
    g_hi = G.astype(np.float32).astype(NPF8)
    g_lo = (G - g_hi.astype(np.float64)).astype(np.float32).astype(NPF8)
    # [i_lo, b, i_hi, hl, q] with i = i_hi*128 + i_lo
    g8 = np.ascontiguousarray(
        np.stack([g_hi, g_lo], axis=2)          # [B, 256, 2, 256]
        .reshape(B, 2, 128, 2, 256).transpose(2, 0, 1, 3, 4))

    xpad = np.zeros((B, LK_PAD, 258), np.float32)
    xpad[:, :LK, :256] = input
    xpad[:, :LK, 256] = 1.0   # ones-column -> denom; 0 on padded rows
    x8 = xpad.astype(NPF8)    # [B, LK_PAD, 258]

    in_maps = []
    for cid in range(N_CORES):
        sl = x8[:, cid * KS:(cid + 1) * KS, :]
        # natural: [B, 128(part), NSUB, 258]; node k = t*128 + p
        xn_c = sl.reshape(B, NSUB, 128, 258).transpose(0, 2, 1, 3)
        # transposed DoubleRow: [B, 128(i_lo), 2(i_hi), KS]
        xt_c = np.ascontiguousarray(
            sl[:, :, :256].transpose(0, 2, 1)).reshape(B, 2, 128, KS)
        xt_c = xt_c.transpose(0, 2, 1, 3)
        in_maps.append({
            "xn": np.ascontiguousarray(xn_c),
            "xt": np.ascontiguousarray(xt_c),
            "g": g8,
        })
    return in_maps


def kernel(query, input, Wq, bq, Wk, bk, Wv, bv):
    # bk provably cancels in softmax over k; bq folds into G; bv is applied
    # in the host epilogue.
    query = np.asarray(query, dtype=np.float32)
    input = np.asarray(input, dtype=np.float32)
    Wq = np.asarray(Wq, dtype=np.float32)
    bq = np.asarray(bq, dtype=np.float32)
    Wk = np.asarray(Wk, dtype=np.float32)
    Wv = np.asarray(Wv, dtype=np.float32)
    bv = np.asarray(bv, dtype=np.float32)

    nc = build()
    in_maps = _prepare_inputs(query, input, Wq, bq, Wk)
    res = run_bass_kernel_spmd(nc, in_maps, core_ids=list(range(N_CORES)))
    kernel._last_result = res

    numer = np.zeros((B, OUT))
    denom = np.zeros((B, OUT))
    Wv64 = Wv.astype(np.float64)
    for r in res.results:
        H = r["ht"].astype(np.float64).reshape(B, OUT, 257)  # j = h*128 + p
        numer += (H[:, :, :256] * Wv64[None]).sum(axis=2)
        denom += H[:, :, 256]
    out = numer / denom + bv
    return out.astype(np.float32)


if __name__ == "__main__":
    # CoreSim smoke test on the full module with random fp8 inputs.
    from concourse.bass_interp import CoreSim

    rng = np.random.default_rng(0)
    xpad = np.zeros((B, KS, 258), np.float32)
    xpad[:, :, :256] = rng.standard_normal((B, KS, 256))
    xpad[:, :, 256] = 1.0
    x8 = xpad.astype(NPF8)
    xn_np = np.ascontiguousarray(
        x8.reshape(B, NSUB, 128, 258).transpose(0, 2, 1, 3))
    xt_np = np.ascontiguousarray(
        x8[:, :, :256].transpose(0, 2, 1)).reshape(B, 2, 128, KS)
    xt_np = np.ascontiguousarray(xt_np.transpose(0, 2, 1, 3))
    g_np = (rng.standard_normal((B, 256, 256)) * 1.8).astype(np.float64)
    g_hi = g_np.astype(np.float32).astype(NPF8)
    g_lo = (g_np - g_hi.astype(np.float64)).astype(np.float32).astype(NPF8)
    g8 = np.ascontiguousarray(
        np.stack([g_hi, g_lo], axis=2)
        .reshape(B, 2, 128, 2, 256).transpose(2, 0, 1, 3, 4))

    nc = build()
    sim = CoreSim(nc)
    sim.tensor("xn")[:] = xn_np
    sim.tensor("xt")[:] = xt_np
    sim.tensor("g")[:] = g8
    sim.simulate()
    got = np.array(sim.tensor("ht")).astype(np.float64).reshape(B, OUT, 257)

    x32 = x8.astype(np.float32)
    gsum = (g8[:, :, :, 0, :].astype(np.float32)
            + g8[:, :, :, 1, :].astype(np.float32))  # [il, b, ih, q]
    want = np.zeros((B, OUT, 257))
    for b in range(B):
        gb = gsum[:, b, :, :].transpose(1, 0, 2).reshape(256, 256)
        s = x32[b, :, :256] @ gb  # = SG * scores
        # per-subtile engine assignment
        e = np.zeros((KS, 256), np.float32)
        for t in range(NSUB):
            grp_i = t // GRP if t < NGRP * GRP else None
            rows = slice(t * 128, (t + 1) * 128)
            # note: node k = t*128+p lives at partition p, subtile t; scores
            # rows here are k-major which matches
            if grp_i is not None and grp_i in ACT_GROUPS:
                e[rows] = np.exp(s[rows] / SG).astype(NPF8).astype(np.float32)
            else:
                y = np.trunc(A8 * s[rows] + B8).astype(np.int8)
                e[rows] = y.view(NPF8).astype(np.float32)
        want[b] = e.T @ x32[b, :, :257]
    err = np.abs(got - want).max() / np.abs(want).max()
    print("CoreSim rel err vs bit-exact model:", err)
    assert err < 1e-3, err
    print("OK")


# revision 60
# speedup vs baseline: 1.1350x; 1.1350x over previous
"""Trainium2 Bass kernel for nn_Attention_9122510537215 (gnn_message_passing).

Math (per batch b):
    Q = query @ Wq.T + bq                  [LQ=256, 256]
    K = input @ Wk.T + bk                  [LK, 256]
    V = input @ Wv.T + bv                  [LK, 256]
    alpha = softmax_k(Q @ K.T / 16)        [256, LK]
    out[j] = sum_k alpha[j, k] * V[k, j]   [256]

Algebraic restructuring:
  * bk shifts every score column by a constant along k -> cancels in softmax_k.
  * G[b] = Wk.T @ (query_b @ Wq.T + bq).T / 16, so scoresT = input @ G  ([LK, 256]).
  * Instead of materializing V, accumulate H[j, i] = sum_k e[k, j] * input[k, i]
    (e = exp(scores)); numer[j] = sum_i H[j, i] * Wv[j, i]; an appended
    ones-column yields denom[j] = H[j, 256]; out = numer / denom + bv.
  * Softmax is computed unnormalized without max-subtraction (scores are O(1)).

Performance structure (vs the fp16 predecessor):
  * All matmuls run in fp8e4 (e4m3) with MatmulPerfMode.DoubleRow: two
    128-row contraction tiles per pass at 0.5 cycles/output-column.
      - scores: per 128-node subtile, ONE DR matmul contracts all 256 input
        features (xt laid out [i_lo(128 part), 2(i_hi), k]).
      - H: subtile PAIRS contract 256 nodes per pass (e laid out
        [k(128 part), 2(pair), j]; xn natural [k, 2(pair), 257]).
  * exp is the serial bottleneck (B*LQ*LK/8 = 6.4M exps/core), so it is
    SPLIT across two engines: ScalarE computes exact Exp (fp8 out,
    scale=1/SG), and VectorE computes a Schraudolph-style exp by writing
    round(A8*score + B8) as int8 and BITCASTING those bytes as fp8e4
    (weights' 3.3% rms wiggle averages out over 50k softmax terms).
  * Whole per-core input (12.9 MB fp8) is SBUF-resident; chunked DMAs on a
    single ordered sync queue overlap the whole compute pipeline.
  * Distribution: LK padded to 50176 = 8*6272, sharded over 8 cores; each
    core returns fp16 partial H [B, 2, 128, 257]; host reduces in float64.
"""

import numpy as np
from contextlib import ExitStack

import concourse.bass as bass
import concourse.mybir as mybir
import concourse.tile as tile
from concourse import bacc
from concourse.bass_utils import run_bass_kernel_spmd

# Problem constants (hardcoded; kernel.py must be self-contained).
B = 4
LQ = 256
LK = 50000
OUT = 256
KV = 256            # input feature dim
NORM = 1.0 / 16.0   # 1/sqrt(OUT)

N_CORES = 8
SUB = 128                  # nodes per subtile (PE contraction width)
NSUB = 49                  # subtiles per core per batch
KS = NSUB * SUB            # 6272 nodes per core per batch
LK_PAD = KS * N_CORES      # 50176
GRP = 4                    # subtiles per exp/psum group (2 DoubleRow pairs,
NGRP = NSUB // GRP         # 2 PSUM banks) -> 12 groups; subtile 48 is odd
CHUNKS = (12, 12, 12, 8, 5)   # subtiles per DMA chunk (contiguous ranges)
NCHUNK = len(CHUNKS)
CH_OFF = tuple(sum(CHUNKS[:i]) for i in range(NCHUNK))
# Load order: the small tail chunk (groups g11 + odd subtile 48) comes
# FIRST so the odd group's serial exp chain overlaps the batch body
# instead of extending the batch tail.
CHUNK_ORDER = (4, 0, 1, 2, 3)
GROUP_ORDER = (11, 12, 0, 1, 2, 3, 4, 5, 6, 7, 8, 9, 10)  # 12 = odd group

SG = 64.0                        # score scale inside PSUM (folded into g)
A8 = 8 * np.log2(np.e) / SG      # Schraudolph fp8e4 slope
B8 = 56.05                       # 8*7 bias, +0.5 trunc->round, -0.45 mean-cal

# exp engine per group index: ScalarE / VectorE (Schraudolph), alternating
# in STREAM order (GROUP_ORDER) so the two exp engines run concurrently.
ACT_GROUPS = frozenset((11, 12, 1, 3, 5, 7, 9))   # g11, odd, then alternate
POOL_GROUPS = frozenset()
ODD_ON_ACT = True  # kept for the __main__ reference model
PEND = 3                   # H matmuls lag their group by 3 so the PE FIFO
                           # never blocks upcoming scores on a pending exp

F8 = mybir.dt.float8e4
F16 = mybir.dt.float16
F32 = mybir.dt.float32
I8 = mybir.dt.int8
NPF8 = mybir.dt.np(mybir.dt.float8e4)
DR = mybir.MatmulPerfMode.DoubleRow


def _chunk_of(s):
    """DMA chunk index and subtile offset within the chunk."""
    for c in range(NCHUNK - 1, -1, -1):
        if s >= CH_OFF[c]:
            return c, s - CH_OFF[c]
    raise AssertionError(s)


def build(mode="full", pend_depth=PEND, flush_per_batch=False,
          g_first=False, merge_out=True, xt_lookahead=0):
    """Emit the per-core SPMD Bass module (identical on all cores).

    mode: "full" | "noh" (skip H matmuls+drain) | "noglo" (single G matmul)
    | "noexp" (static e tile; tests DMA+PE pipeline) — ablations for timing.
    """
    nc = bacc.Bacc("TRN2", target_bir_lowering=False, debug=False,
                   num_devices=N_CORES)
    xn = nc.dram_tensor("xn", [B, 128, NSUB, 258], F8, kind="ExternalInput")
    xt = nc.dram_tensor("xt", [B, 128, 2, KS], F8, kind="ExternalInput")
    # g[..., 0, :] = fp8(G), g[..., 1, :] = fp8(G - fp8(G)): the second,
    # accumulating matmul restores ~11-bit G precision. (A first-order host
    # correction does NOT work: the fp8 e-quantization's 12.5% buckets turn
    # the smooth score shift into quasi-random bucket flips.)
    g = nc.dram_tensor("g", [128, B, 2, 2, 256], F8, kind="ExternalInput")
    ht = nc.dram_tensor("ht", [B, 128, 2, 257], F16, kind="ExternalOutput")

    with ExitStack() as ctx:
        tc = ctx.enter_context(tile.TileContext(nc))
        gp = ctx.enter_context(tc.tile_pool(name="gp", bufs=1))
        xnp = ctx.enter_context(tc.tile_pool(name="xnp", bufs=B * NCHUNK))
        xtp = ctx.enter_context(tc.tile_pool(name="xtp", bufs=B * NCHUNK))
        ep = ctx.enter_context(tc.tile_pool(name="ep", bufs=4))
        eip = ctx.enter_context(tc.tile_pool(name="eip", bufs=4))
        hop = ctx.enter_context(tc.tile_pool(name="hop", bufs=4))
        spp = ctx.enter_context(tc.tile_pool(name="spp", bufs=3, space="PSUM"))
        hpp = ctx.enter_context(tc.tile_pool(name="hpp", bufs=1, space="PSUM"))

        # G hi+lo, resident: [i_lo(128 part), b, i_hi, hl, q]; loaded per
        # batch so batch 0's scores aren't gated on the whole tensor.
        g_sb = gp.tile([128, B, 2, 2, 256], F8)

        # Chunked input loads on one ordered queue (sync/SP), interleaved so
        # batch 0's first chunks land first; xt precedes xn since scores
        # only need xt+g.
        xn_tiles = {}
        xt_tiles = {}
        cmax = max(CHUNKS)
        def load_xt(b, c):
            s0, ns = CH_OFF[c], CHUNKS[c]
            xtt = xtp.tile([128, 2, cmax * SUB], F8, tag="xt")
            nc.sync.dma_start(
                out=xtt[:, :, :ns * SUB],
                in_=xt[b, :, :, s0 * SUB:(s0 + ns) * SUB])
            xt_tiles[(b, c)] = xtt

        def load_xn(b, c):
            s0, ns = CH_OFF[c], CHUNKS[c]
            xnt = xnp.tile([128, cmax, 258], F8, tag="xn")
            nc.sync.dma_start(out=xnt[:, :ns, :], in_=xn[b, :, s0:s0 + ns, :])
            xn_tiles[(b, c)] = xnt

        for b in range(B):
            if g_first:
                nc.sync.dma_start(out=g_sb[:, b], in_=g[:, b])
            order = list(CHUNK_ORDER)
            la = xt_lookahead
            for ci, c in enumerate(order):
                load_xt(b, c)
                if not g_first and ci == 1:
                    nc.sync.dma_start(out=g_sb[:, b], in_=g[:, b])
                if ci >= la:
                    load_xn(b, order[ci - la])
            for ci in range(len(order) - la, len(order)):
                load_xn(b, order[ci])

        # groups: (subtile list, engine) per batch, in GROUP_ORDER.
        def eng_of(g_):
            if g_ in POOL_GROUPS:
                return "pool"
            return "act" if g_ in ACT_GROUPS else "dve"
        def subs_of(g_):
            if g_ == NGRP:  # odd tail group
                return [NSUB - 1]
            return [g_ * GRP + i for i in range(GRP)]
        groups = [(subs_of(g_), eng_of(g_)) for g_ in GROUP_ORDER]
        # ONE global stream of (batch, group) — no per-batch pend flush, so
        # batch b+1's scores interleave with batch b's trailing H matmuls
        # and the exp engines never drain at batch boundaries.
        stream = [(b, sub, eng) for b in range(B) for sub, eng in groups]

        ht_acc = {}       # b -> (ht0, ht1)
        pend = []         # (b, e_f8, subs) whose H matmuls are not yet issued
        popped = [0] * B  # H groups issued per batch

        e_static = None
        if mode == "noexp":
            stp = ctx.enter_context(tc.tile_pool(name="stp", bufs=1))
            e_static = stp.tile([128, GRP, 256], F8, tag="es")
            nc.vector.memset(e_static[:, :, :], 1.0)

        def pop_h():
            b, pe_, psubs = pend.pop(0)
            if b not in ht_acc:
                ht_acc[b] = (hpp.tile([128, 257], F32, tag="ht0", name="ht0"),
                             hpp.tile([128, 257], F32, tag="ht1", name="ht1"))
            ht0, ht1 = ht_acc[b]
            first_grp = popped[b] == 0
            popped[b] += 1
            is_last_grp = popped[b] == len(groups)
            npair = len(psubs) // 2
            for i in range(npair):
                s = psubs[2 * i]
                c, off = _chunk_of(s)
                xnt = xn_tiles[(b, c)]
                first = first_grp and i == 0
                last = (is_last_grp and i == npair - 1
                        and len(psubs) % 2 == 0)
                for h, hacc in ((0, ht0), (1, ht1)):
                    nc.tensor.matmul(
                        hacc[:, :],
                        pe_[:, 2 * i:2 * i + 2, h * 128:(h + 1) * 128],
                        xnt[:, off:off + 2, 0:257],
                        start=first, stop=last, perf_mode=DR)
            if len(psubs) % 2:  # odd tail: plain fp8 matmul
                s = psubs[-1]
                c, off = _chunk_of(s)
                xnt = xn_tiles[(b, c)]
                i = len(psubs) - 1
                for h, hacc in ((0, ht0), (1, ht1)):
                    nc.tensor.matmul(
                        hacc[:, :],
                        pe_[:, i, h * 128:(h + 1) * 128],
                        xnt[:, off, 0:257],
                        start=first_grp and npair == 0, stop=is_last_grp)
            if is_last_grp:  # drain on both engines in parallel, out via SP
                hts = hop.tile([128, 2, 257], F16)
                nc.scalar.copy(hts[:, 0, :], ht0[:, :])
                nc.vector.tensor_copy(hts[:, 1, :], ht1[:, :])
                if merge_out:
                    nc.sync.dma_start(out=ht[b], in_=hts[:, :, :])
                else:
                    nc.sync.dma_start(out=ht[b, :, 0], in_=hts[:, 0, :])
                    nc.sync.dma_start(out=ht[b, :, 1], in_=hts[:, 1, :])

        for b, subs, eng in stream:
            sz = len(subs)
            sp = spp.tile([128, GRP, 256], F32)
            nglo = 1 if mode == "noglo" else 2
            for i in range(sz):
                c, off = _chunk_of(subs[i])
                xtt = xt_tiles[(b, c)]
                for hl in range(nglo):  # G hi then lo residual
                    nc.tensor.matmul(
                        sp[:, i, :],
                        xtt[:, :, off * SUB:(off + 1) * SUB],
                        g_sb[:, b, :, hl, :],
                        start=hl == 0, stop=hl == nglo - 1, perf_mode=DR)
            if mode == "noexp":
                e_f8 = e_static
            elif eng == "act":
                e = ep.tile([128, GRP, 256], F8, tag="ea")
                nc.scalar.activation(
                    e[:, :sz, :], sp[:, :sz, :],
                    mybir.ActivationFunctionType.Exp, scale=1.0 / SG)
                e_f8 = e
            else:
                ei = eip.tile([128, GRP, 256], I8, tag="ei")
                veng = nc.vector if eng == "dve" else nc.gpsimd
                veng.tensor_scalar(
                    ei[:, :sz, :], sp[:, :sz, :], A8, B8,
                    mybir.AluOpType.mult, mybir.AluOpType.add)
                e_f8 = ei[:, :, :].bitcast(F8)
            if mode != "noh":
                pend.append((b, e_f8, subs))
                # H matmuls lag pend_depth groups behind so the PE FIFO
                # never blocks upcoming scores on a pending exp.
                if len(pend) > pend_depth:
                    pop_h()
                if flush_per_batch and subs is groups[-1][0]:
                    while pend:
                        pop_h()
        while pend:
            pop_h()
    nc.compile()
    return nc


def _prepare_inputs(query, input, Wq, bq, Wk):
    """Host marshalling: folded G + fp8 input in both layouts, k-sharded.

    """
    # G[b] = Wk.T @ (query_b @ Wq.T + bq).T -> [B, 256(i), 256(q)], then
    # * NORM (1/16) * SG (64) so PSUM scores arrive pre-scaled by SG.
    Q = query.astype(np.float64) @ Wq.T.astype(np.float64) + bq
    G = np.einsum('di,bqd->biq', Wk.astype(np.float64), Q) * (NORM * SG)
    g_hi = G.astype(np.float32).astype(NPF8)
    g_lo = (G - g_hi.astype(np.float64)).astype(np.float32).astype(NPF8)
    # [i_lo, b, i_hi, hl, q] with i = i_hi*128 + i_lo
    g8 = np.ascontiguousarray(
        np.stack([g_hi, g_lo], axis=2)          # [B, 256, 2, 256]
        .reshape(B, 2, 128, 2, 256).transpose(2, 0, 1, 3, 4))

    xpad = np.zeros((B, LK_PAD, 258), np.float32)
    xpad[:, :LK, :256] = input
    xpad[:, :LK, 256] = 1.0   # ones-column -> denom; 0 on padded rows
    x8 = xpad.astype(NPF8)    # [B, LK_PAD, 258]

    in_maps = []
    for cid in range(N_CORES):
        sl = x8[:, cid * KS:(cid + 1) * KS, :]
        # natural: [B, 128(part), NSUB, 258]; node k = t*128 + p
        xn_c = sl.reshape(B, NSUB, 128, 258).transpose(0, 2, 1, 3)
        # transposed DoubleRow: [B, 128(i_lo), 2(i_hi), KS]
        xt_c = np.ascontiguousarray(
            sl[:, :, :256].transpose(0, 2, 1)).reshape(B, 2, 128, KS)
        xt_c = xt_c.transpose(0, 2, 1, 3)
        in_maps.append({
            "xn": np.ascontiguousarray(xn_c),
            "xt": np.ascontiguousarray(xt_c),
            "g": g8,
        })
    return in_maps


def kernel(query, input, Wq, bq, Wk, bk, Wv, bv):
    # bk provably cancels in softmax over k; bq folds into G; bv is applied
    # in the host epilogue.
    query = np.asarray(query, dtype=np.float32)
    input = np.asarray(input, dtype=np.float32)
    Wq = np.asarray(Wq, dtype=np.float32)
    bq = np.asarray(bq, dtype=np.float32)
    Wk = np.asarray(Wk, dtype=np.float32)
    Wv = np.asarray(Wv, dtype=np.float32)
    bv = np.asarray(bv, dtype=np.float32)

    nc = build()
    in_maps = _prepare_inputs(query, input, Wq, bq, Wk)
    res = run_bass_kernel_spmd(nc, in_maps, core_ids=list(range(N_CORES)))
    kernel._last_result = res

    numer = np.zeros((B, OUT))
    denom = np.zeros((B, OUT))
    Wv64 = Wv.astype(np.float64)
    for r in res.results:
        # [B, 128(p), 2(h), 257] -> [B, 256(j), 257] with j = h*128 + p
        H = (r["ht"].astype(np.float64)
             .transpose(0, 2, 1, 3).reshape(B, OUT, 257))
        numer += (H[:, :, :256] * Wv64[None]).sum(axis=2)
        denom += H[:, :, 256]
    out = numer / denom + bv
    return out.astype(np.float32)


if __name__ == "__main__":
    # CoreSim smoke test on the full module with random fp8 inputs.
    from concourse.bass_interp import CoreSim

    rng = np.random.default_rng(0)
    xpad = np.zeros((B, KS, 258), np.float32)
    xpad[:, :, :256] = rng.standard_normal((B, KS, 256))
    xpad[:, :, 256] = 1.0
    x8 = xpad.astype(NPF8)
    xn_np = np.ascontiguousarray(
        x8.reshape(B, NSUB, 128, 258).transpose(0, 2, 1, 3))
    xt_np = np.ascontiguousarray(
        x8[:, :, :256].transpose(0, 2, 1)).reshape(B, 2, 128, KS)
    xt_np = np.ascontiguousarray(xt_np.transpose(0, 2, 1, 3))
    g_np = (rng.standard_normal((B, 256, 256)) * 1.8).astype(np.float64)
    g_hi = g_np.astype(np.float32).astype(NPF8)
    g_lo = (g_np - g_hi.astype(np.float64)).astype(np.float32).astype(NPF8)
    g8 = np.ascontiguousarray(
        np.stack([g_hi, g_lo], axis=2)
        .reshape(B, 2, 128, 2, 256).transpose(2, 0, 1, 3, 4))

    nc = build()
    sim = CoreSim(nc)
    sim.tensor("xn")[:] = xn_np
    sim.tensor("xt")[:] = xt_np
    sim.tensor("g")[:] = g8
    sim.simulate()
    got = (np.array(sim.tensor("ht")).astype(np.float64)
           .transpose(0, 2, 1, 3).reshape(B, OUT, 257))

    x32 = x8.astype(np.float32)
    gsum = (g8[:, :, :, 0, :].astype(np.float32)
            + g8[:, :, :, 1, :].astype(np.float32))  # [il, b, ih, q]
    want = np.zeros((B, OUT, 257))
    for b in range(B):
        gb = gsum[:, b, :, :].transpose(1, 0, 2).reshape(256, 256)
        s = x32[b, :, :256] @ gb  # = SG * scores
        # per-subtile engine assignment
        e = np.zeros((KS, 256), np.float32)
        for t in range(NSUB):
            grp_i = t // GRP if t < NGRP * GRP else NGRP
            rows = slice(t * 128, (t + 1) * 128)
            # note: node k = t*128+p lives at partition p, subtile t; scores
            # rows here are k-major which matches
            if grp_i in ACT_GROUPS:
                e[rows] = np.exp(s[rows] / SG).astype(NPF8).astype(np.float32)
            else:
                y = np.trunc(A8 * s[rows] + B8).astype(np.int8)
                e[rows] = y.view(NPF8).astype(np.float32)
        want[b] = e.T @ x32[b, :, :257]
    err = np.abs(got - want).max() / np.abs(want).max()
    print("CoreSim rel err vs bit-exact model:", err)
    assert err < 1e-3, err
    print("OK")


# revision 65
# speedup vs baseline: 1.1601x; 1.0221x over previous
"""Trainium2 Bass kernel for nn_Attention_9122510537215 (gnn_message_passing).

Math (per batch b):
    Q = query @ Wq.T + bq                  [LQ=256, 256]
    K = input @ Wk.T + bk                  [LK, 256]
    V = input @ Wv.T + bv                  [LK, 256]
    alpha = softmax_k(Q @ K.T / 16)        [256, LK]
    out[j] = sum_k alpha[j, k] * V[k, j]   [256]

Algebraic restructuring:
  * bk shifts every score column by a constant along k -> cancels in softmax_k.
  * G[b] = Wk.T @ (query_b @ Wq.T + bq).T / 16, so scoresT = input @ G  ([LK, 256]).
  * Instead of materializing V, accumulate H[j, i] = sum_k e[k, j] * input[k, i]
    (e = exp(scores)); numer[j] = sum_i H[j, i] * Wv[j, i]; an appended
    ones-column yields denom[j] = H[j, 256]; out = numer / denom + bv.
  * Softmax is computed unnormalized without max-subtraction (scores are O(1)).

Performance structure (vs the fp16 predecessor):
  * All matmuls run in fp8e4 (e4m3) with MatmulPerfMode.DoubleRow: two
    128-row contraction tiles per pass at 0.5 cycles/output-column.
      - scores: per 128-node subtile, ONE DR matmul contracts all 256 input
        features (xt laid out [i_lo(128 part), 2(i_hi), k]).
      - H: subtile PAIRS contract 256 nodes per pass (e laid out
        [k(128 part), 2(pair), j]; xn natural [k, 2(pair), 257]).
  * exp is the serial bottleneck (B*LQ*LK/8 = 6.4M exps/core), so it is
    SPLIT across two engines: ScalarE computes exact Exp (fp8 out,
    scale=1/SG), and VectorE computes a Schraudolph-style exp by writing
    round(A8*score + B8) as int8 and BITCASTING those bytes as fp8e4
    (weights' 3.3% rms wiggle averages out over 50k softmax terms).
  * Whole per-core input (12.9 MB fp8) is SBUF-resident; chunked DMAs on a
    single ordered sync queue overlap the whole compute pipeline.
  * Distribution: LK padded to 50176 = 8*6272, sharded over 8 cores; each
    core returns fp16 partial H [B, 2, 128, 257]; host reduces in float64.
"""

import numpy as np
from contextlib import ExitStack

import concourse.bass as bass
import concourse.mybir as mybir
import concourse.tile as tile
from concourse import bacc
from concourse.bass_utils import run_bass_kernel_spmd

# Problem constants (hardcoded; kernel.py must be self-contained).
B = 4
LQ = 256
LK = 50000
OUT = 256
KV = 256            # input feature dim
NORM = 1.0 / 16.0   # 1/sqrt(OUT)

N_CORES = 8
SUB = 128                  # nodes per subtile (PE contraction width)
NSUB = 49                  # subtiles per core per batch
KS = NSUB * SUB            # 6272 nodes per core per batch
LK_PAD = KS * N_CORES      # 50176
GRP = 4                    # subtiles per exp/psum group (2 DoubleRow pairs,
NGRP = NSUB // GRP         # 2 PSUM banks) -> 12 groups; subtile 48 is odd
CHUNKS = (12, 12, 12, 8, 5)   # subtiles per DMA chunk (contiguous ranges)
NCHUNK = len(CHUNKS)
CH_OFF = tuple(sum(CHUNKS[:i]) for i in range(NCHUNK))
# Load order: the small tail chunk (groups g11 + odd subtile 48) comes
# FIRST so the odd group's serial exp chain overlaps the batch body
# instead of extending the batch tail.
CHUNK_ORDER = (4, 0, 1, 2, 3)
GROUP_ORDER = (11, 12, 0, 1, 2, 3, 4, 5, 6, 7, 8, 9, 10)  # 12 = odd group

SG = 64.0                        # score scale inside PSUM (folded into g)
A8 = 8 * np.log2(np.e) / SG      # Schraudolph fp8e4 slope
B8 = 56.05                       # 8*7 bias, +0.5 trunc->round, -0.45 mean-cal

# exp engine per group index: ScalarE / VectorE (Schraudolph), alternating
# in STREAM order (GROUP_ORDER) so the two exp engines run concurrently.
ACT_GROUPS = frozenset((12, 0, 2, 4, 6, 8, 10))   # odd + evens; g11 on DVE
POOL_GROUPS = frozenset()
ODD_ON_ACT = True  # kept for the __main__ reference model
PEND = 3                   # H matmuls lag their group by 3 so the PE FIFO
                           # never blocks upcoming scores on a pending exp

F8 = mybir.dt.float8e4
F16 = mybir.dt.float16
F32 = mybir.dt.float32
I8 = mybir.dt.int8
NPF8 = mybir.dt.np(mybir.dt.float8e4)
DR = mybir.MatmulPerfMode.DoubleRow


def _chunk_of(s):
    """DMA chunk index and subtile offset within the chunk."""
    for c in range(NCHUNK - 1, -1, -1):
        if s >= CH_OFF[c]:
            return c, s - CH_OFF[c]
    raise AssertionError(s)


def build(mode="full", pend_depth=PEND, flush_per_batch=False,
          g_first=False, merge_out=True, xt_lookahead=0):
    """Emit the per-core SPMD Bass module (identical on all cores).

    mode: "full" | "noh" (skip H matmuls+drain) | "noglo" (single G matmul)
    | "noexp" (static e tile; tests DMA+PE pipeline) — ablations for timing.
    """
    nc = bacc.Bacc("TRN2", target_bir_lowering=False, debug=False,
                   num_devices=N_CORES)
    xn = nc.dram_tensor("xn", [B, 128, NSUB, 258], F8, kind="ExternalInput")
    xt = nc.dram_tensor("xt", [B, 128, 2, KS], F8, kind="ExternalInput")
    # g[..., 0, :] = fp8(G), g[..., 1, :] = fp8(G - fp8(G)): the second,
    # accumulating matmul restores ~11-bit G precision. (A first-order host
    # correction does NOT work: the fp8 e-quantization's 12.5% buckets turn
    # the smooth score shift into quasi-random bucket flips.)
    g = nc.dram_tensor("g", [128, B, 2, 2, 256], F8, kind="ExternalInput")
    ht = nc.dram_tensor("ht", [B, 128, 2, 257], F16, kind="ExternalOutput")

    with ExitStack() as ctx:
        tc = ctx.enter_context(tile.TileContext(nc))
        gp = ctx.enter_context(tc.tile_pool(name="gp", bufs=1))
        xnp = ctx.enter_context(tc.tile_pool(name="xnp", bufs=B * NCHUNK))
        xtp = ctx.enter_context(tc.tile_pool(name="xtp", bufs=B * NCHUNK))
        ep = ctx.enter_context(tc.tile_pool(name="ep", bufs=4))
        eip = ctx.enter_context(tc.tile_pool(name="eip", bufs=4))
        hop = ctx.enter_context(tc.tile_pool(name="hop", bufs=4))
        spp = ctx.enter_context(tc.tile_pool(name="spp", bufs=3, space="PSUM"))
        hpp = ctx.enter_context(tc.tile_pool(name="hpp", bufs=1, space="PSUM"))

        # G hi+lo, resident: [i_lo(128 part), b, i_hi, hl, q]; loaded per
        # batch so batch 0's scores aren't gated on the whole tensor.
        g_sb = gp.tile([128, B, 2, 2, 256], F8)

        # Chunked input loads on one ordered queue (sync/SP), interleaved so
        # batch 0's first chunks land first; xt precedes xn since scores
        # only need xt+g.
        xn_tiles = {}
        xt_tiles = {}
        cmax = max(CHUNKS)
        def load_xt(b, c):
            s0, ns = CH_OFF[c], CHUNKS[c]
            xtt = xtp.tile([128, 2, cmax * SUB], F8, tag="xt")
            nc.sync.dma_start(
                out=xtt[:, :, :ns * SUB],
                in_=xt[b, :, :, s0 * SUB:(s0 + ns) * SUB])
            xt_tiles[(b, c)] = xtt

        def load_xn(b, c):
            s0, ns = CH_OFF[c], CHUNKS[c]
            xnt = xnp.tile([128, cmax, 258], F8, tag="xn")
            nc.sync.dma_start(out=xnt[:, :ns, :], in_=xn[b, :, s0:s0 + ns, :])
            xn_tiles[(b, c)] = xnt

        for b in range(B):
            if g_first:
                nc.sync.dma_start(out=g_sb[:, b], in_=g[:, b])
            order = list(CHUNK_ORDER)
            la = xt_lookahead
            for ci, c in enumerate(order):
                load_xt(b, c)
                if not g_first and ci == 1:
                    nc.sync.dma_start(out=g_sb[:, b], in_=g[:, b])
                if ci >= la:
                    load_xn(b, order[ci - la])
            for ci in range(len(order) - la, len(order)):
                load_xn(b, order[ci])

        # groups: (subtile list, engine) per batch, in GROUP_ORDER.
        def eng_of(g_):
            if g_ in POOL_GROUPS:
                return "pool"
            return "act" if g_ in ACT_GROUPS else "dve"
        def subs_of(g_):
            if g_ == NGRP:  # odd tail group
                return [NSUB - 1]
            return [g_ * GRP + i for i in range(GRP)]
        groups = [(subs_of(g_), eng_of(g_)) for g_ in GROUP_ORDER]
        # ONE global stream of (batch, group) — no per-batch pend flush, so
        # batch b+1's scores interleave with batch b's trailing H matmuls
        # and the exp engines never drain at batch boundaries.
        stream = [(b, sub, eng) for b in range(B) for sub, eng in groups]

        ht_acc = {}       # b -> (ht0, ht1)
        pend = []         # (b, e_f8, subs) whose H matmuls are not yet issued
        popped = [0] * B  # H groups issued per batch

        e_static = None
        if mode == "noexp":
            stp = ctx.enter_context(tc.tile_pool(name="stp", bufs=1))
            e_static = stp.tile([128, GRP, 256], F8, tag="es")
            nc.vector.memset(e_static[:, :, :], 1.0)

        def pop_h():
            b, pe_, psubs = pend.pop(0)
            if b not in ht_acc:
                ht_acc[b] = (hpp.tile([128, 257], F32, tag="ht0", name="ht0"),
                             hpp.tile([128, 257], F32, tag="ht1", name="ht1"))
            ht0, ht1 = ht_acc[b]
            first_grp = popped[b] == 0
            popped[b] += 1
            is_last_grp = popped[b] == len(groups)
            npair = len(psubs) // 2
            for i in range(npair):
                s = psubs[2 * i]
                c, off = _chunk_of(s)
                xnt = xn_tiles[(b, c)]
                first = first_grp and i == 0
                last = (is_last_grp and i == npair - 1
                        and len(psubs) % 2 == 0)
                for h, hacc in ((0, ht0), (1, ht1)):
                    nc.tensor.matmul(
                        hacc[:, :],
                        pe_[:, 2 * i:2 * i + 2, h * 128:(h + 1) * 128],
                        xnt[:, off:off + 2, 0:257],
                        start=first, stop=last, perf_mode=DR)
            if len(psubs) % 2:  # odd tail: plain fp8 matmul
                s = psubs[-1]
                c, off = _chunk_of(s)
                xnt = xn_tiles[(b, c)]
                i = len(psubs) - 1
                for h, hacc in ((0, ht0), (1, ht1)):
                    nc.tensor.matmul(
                        hacc[:, :],
                        pe_[:, i, h * 128:(h + 1) * 128],
                        xnt[:, off, 0:257],
                        start=first_grp and npair == 0, stop=is_last_grp)
            if is_last_grp:  # drain on both engines in parallel, out via SP
                hts = hop.tile([128, 2, 257], F16)
                nc.scalar.copy(hts[:, 0, :], ht0[:, :])
                nc.vector.tensor_copy(hts[:, 1, :], ht1[:, :])
                if merge_out:
                    nc.sync.dma_start(out=ht[b], in_=hts[:, :, :])
                else:
                    nc.sync.dma_start(out=ht[b, :, 0], in_=hts[:, 0, :])
                    nc.sync.dma_start(out=ht[b, :, 1], in_=hts[:, 1, :])

        for b, subs, eng in stream:
            sz = len(subs)
            sp = spp.tile([128, GRP, 256], F32)
            nglo = 1 if mode == "noglo" else 2
            for i in range(sz):
                c, off = _chunk_of(subs[i])
                xtt = xt_tiles[(b, c)]
                for hl in range(nglo):  # G hi then lo residual
                    nc.tensor.matmul(
                        sp[:, i, :],
                        xtt[:, :, off * SUB:(off + 1) * SUB],
                        g_sb[:, b, :, hl, :],
                        start=hl == 0, stop=hl == nglo - 1, perf_mode=DR)
            if mode == "noexp":
                e_f8 = e_static
            elif eng == "act":
                e = ep.tile([128, GRP, 256], F8, tag="ea")
                nc.scalar.activation(
                    e[:, :sz, :], sp[:, :sz, :],
                    mybir.ActivationFunctionType.Exp, scale=1.0 / SG)
                e_f8 = e
            else:
                ei = eip.tile([128, GRP, 256], I8, tag="ei")
                veng = nc.vector if eng == "dve" else nc.gpsimd
                veng.tensor_scalar(
                    ei[:, :sz, :], sp[:, :sz, :], A8, B8,
                    mybir.AluOpType.mult, mybir.AluOpType.add)
                e_f8 = ei[:, :, :].bitcast(F8)
            if mode != "noh":
                pend.append((b, e_f8, subs))
                # H matmuls lag pend_depth groups behind so the PE FIFO
                # never blocks upcoming scores on a pending exp.
                if len(pend) > pend_depth:
                    pop_h()
                if flush_per_batch and subs is groups[-1][0]:
                    while pend:
                        pop_h()
        while pend:
            pop_h()
    nc.compile()
    return nc


def _prepare_inputs(query, input, Wq, bq, Wk):
    """Host marshalling: folded G + fp8 input in both layouts, k-sharded.

    """
    # G[b] = Wk.T @ (query_b @ Wq.T + bq).T -> [B, 256(i), 256(q)], then
    # * NORM (1/16) * SG (64) so PSUM scores arrive pre-scaled by SG.
    Q = query.astype(np.float64) @ Wq.T.astype(np.float64) + bq
    G = np.einsum('di,bqd->biq', Wk.astype(np.float64), Q) * (NORM * SG)
    g_hi = G.astype(np.float32).astype(NPF8)
    g_lo = (G - g_hi.astype(np.float64)).astype(np.float32).astype(NPF8)
    # [i_lo, b, i_hi, hl, q] with i = i_hi*128 + i_lo
    g8 = np.ascontiguousarray(
        np.stack([g_hi, g_lo], axis=2)          # [B, 256, 2, 256]
        .reshape(B, 2, 128, 2, 256).transpose(2, 0, 1, 3, 4))

    xpad = np.zeros((B, LK_PAD, 258), np.float32)
    xpad[:, :LK, :256] = input
    xpad[:, :LK, 256] = 1.0   # ones-column -> denom; 0 on padded rows
    x8 = xpad.astype(NPF8)    # [B, LK_PAD, 258]

    in_maps = []
    for cid in range(N_CORES):
        sl = x8[:, cid * KS:(cid + 1) * KS, :]
        # natural: [B, 128(part), NSUB, 258]; node k = t*128 + p
        xn_c = sl.reshape(B, NSUB, 128, 258).transpose(0, 2, 1, 3)
        # transposed DoubleRow: [B, 128(i_lo), 2(i_hi), KS]
        xt_c = np.ascontiguousarray(
            sl[:, :, :256].transpose(0, 2, 1)).reshape(B, 2, 128, KS)
        xt_c = xt_c.transpose(0, 2, 1, 3)
        in_maps.append({
            "xn": np.ascontiguousarray(xn_c),
            "xt": np.ascontiguousarray(xt_c),
            "g": g8,
        })
    return in_maps


def kernel(query, input, Wq, bq, Wk, bk, Wv, bv):
    # bk provably cancels in softmax over k; bq folds into G; bv is applied
    # in the host epilogue.
    query = np.asarray(query, dtype=np.float32)
    input = np.asarray(input, dtype=np.float32)
    Wq = np.asarray(Wq, dtype=np.float32)
    bq = np.asarray(bq, dtype=np.float32)
    Wk = np.asarray(Wk, dtype=np.float32)
    Wv = np.asarray(Wv, dtype=np.float32)
    bv = np.asarray(bv, dtype=np.float32)

    nc = build()
    in_maps = _prepare_inputs(query, input, Wq, bq, Wk)
    res = run_bass_kernel_spmd(nc, in_maps, core_ids=list(range(N_CORES)))
    kernel._last_result = res

    numer = np.zeros((B, OUT))
    denom = np.zeros((B, OUT))
    Wv64 = Wv.astype(np.float64)
    for r in res.results:
        # [B, 128(p), 2(h), 257] -> [B, 256(j), 257] with j = h*128 + p
        H = (r["ht"].astype(np.float64)
             .transpose(0, 2, 1, 3).reshape(B, OUT, 257))
        numer += (H[:, :, :256] * Wv64[None]).sum(axis=2)
        denom += H[:, :, 256]
    out = numer / denom + bv
    return out.astype(np.float32)


if __name__ == "__main__":
    # CoreSim smoke test on the full module with random fp8 inputs.
    from concourse.bass_interp import CoreSim

    rng = np.random.default_rng(0)
    xpad = np.zeros((B, KS, 258), np.float32)
    xpad[:, :, :256] = rng.standard_normal((B, KS, 256))
    xpad[:, :, 256] = 1.0
    x8 = xpad.astype(NPF8)
    xn_np = np.ascontiguousarray(
        x8.reshape(B, NSUB, 128, 258).transpose(0, 2, 1, 3))
    xt_np = np.ascontiguousarray(
        x8[:, :, :256].transpose(0, 2, 1)).reshape(B, 2, 128, KS)
    xt_np = np.ascontiguousarray(xt_np.transpose(0, 2, 1, 3))
    g_np = (rng.standard_normal((B, 256, 256)) * 1.8).astype(np.float64)
    g_hi = g_np.astype(np.float32).astype(NPF8)
    g_lo = (g_np - g_hi.astype(np.float64)).astype(np.float32).astype(NPF8)
    g8 = np.ascontiguousarray(
        np.stack([g_hi, g_lo], axis=2)
        .reshape(B, 2, 128, 2, 256).transpose(2, 0, 1, 3, 4))

    nc = build()
    sim = CoreSim(nc)
    sim.tensor("xn")[:] = xn_np
    sim.tensor("xt")[:] = xt_np
    sim.tensor("g")[:] = g8
    sim.simulate()
    got = (np.array(sim.tensor("ht")).astype(np.float64)
           .transpose(0, 2, 1, 3).reshape(B, OUT, 257))

    x32 = x8.astype(np.float32)
    gsum = (g8[:, :, :, 0, :].astype(np.float32)
            + g8[:, :, :, 1, :].astype(np.float32))  # [il, b, ih, q]
    want = np.zeros((B, OUT, 257))
    for b in range(B):
        gb = gsum[:, b, :, :].transpose(1, 0, 2).reshape(256, 256)
        s = x32[b, :, :256] @ gb  # = SG * scores
        # per-subtile engine assignment
        e = np.zeros((KS, 256), np.float32)
        for t in range(NSUB):
            grp_i = t // GRP if t < NGRP * GRP else NGRP
            rows = slice(t * 128, (t + 1) * 128)
            # note: node k = t*128+p lives at partition p, subtile t; scores
            # rows here are k-major which matches
            if grp_i in ACT_GROUPS:
                e[rows] = np.exp(s[rows] / SG).astype(NPF8).astype(np.float32)
            else:
                y = np.trunc(A8 * s[rows] + B8).astype(np.int8)
                e[rows] = y.view(NPF8).astype(np.float32)
        want[b] = e.T @ x32[b, :, :257]
    err = np.abs(got - want).max() / np.abs(want).max()
    print("CoreSim rel err vs bit-exact model:", err)
    assert err < 1e-3, err
    print("OK")


# revision 75
# speedup vs baseline: 1.1642x; 1.0036x over previous
"""Trainium2 Bass kernel for nn_Attention_9122510537215 (gnn_message_passing).

Math (per batch b):
    Q = query @ Wq.T + bq                  [LQ=256, 256]
    K = input @ Wk.T + bk                  [LK, 256]
    V = input @ Wv.T + bv                  [LK, 256]
    alpha = softmax_k(Q @ K.T / 16)        [256, LK]
    out[j] = sum_k alpha[j, k] * V[k, j]   [256]

Algebraic restructuring:
  * bk shifts every score column by a constant along k -> cancels in softmax_k.
  * G[b] = Wk.T @ (query_b @ Wq.T + bq).T / 16, so scoresT = input @ G  ([LK, 256]).
  * Instead of materializing V, accumulate H[j, i] = sum_k e[k, j] * input[k, i]
    (e = exp(scores)); numer[j] = sum_i H[j, i] * Wv[j, i]; an appended
    ones-column yields denom[j] = H[j, 256]; out = numer / denom + bv.
  * Softmax is computed unnormalized without max-subtraction (scores are O(1)).

Performance structure (vs the fp16 predecessor; 108.3us -> 48.1us on the
TimelineSim cost model):
  * All matmuls run in fp8e4 (e4m3) with MatmulPerfMode.DoubleRow: two
    128-row contraction tiles per pass at 0.5 cycles/output-column (4x the
    fp16 FLOP rate in the cost model).
      - scores: per 128-node subtile, TWO accumulating DR matmuls (fp8(G)
        then the residual fp8(G - fp8(G))) contract all 256 input features
        (xt laid out [i_lo(128 part), 2(i_hi), k]). The residual restores
        ~11-bit G precision: G's quantization error interacts with the
        12.5%-wide fp8 e-buckets and is NOT host-correctable to 1st order.
      - H: subtile PAIRS contract 256 nodes per pass (e laid out
        [k(128 part), 2(pair), j]; xn natural [k, 2(pair), 257]).
  * exp is the serial bottleneck (B*LQ*LK/8 = 6.4M exps/core), so it is
    SPLIT across two engines: ScalarE computes exact Exp (fp8 out,
    scale=1/SG), and VectorE computes a Schraudolph-style exp by writing
    trunc(A8*score + B8) as int8 and BITCASTING those bytes as fp8e4
    (weights' ~3.3% rms wiggle averages out over 50k softmax terms).
  * Whole per-core input (13.4 MB fp8) is SBUF-resident; chunked DMAs on a
    single ordered sync queue (aggregate DMA ~360 GB/s is the roofline;
    the per-core payload is 2x the fp8 input because both layouts ship).
  * Pipeline scheduling: ONE global (batch, group) stream — H matmuls lag
    their group's exp by PEND=3 groups so the PE FIFO never blocks the next
    scores on a pending exp; each batch's small tail chunk (g11 + the odd
    49th subtile) is loaded and processed FIRST so the odd group's serial
    chain overlaps the batch body; PSUM = 3 score buffers (6 banks) + the
    2 H accumulators (2 banks).
  * Distribution: LK padded to 50176 = 8*6272, sharded over 8 cores; each
    core returns fp16 partial H [B, 128, 2, 257]; host reduces in float64.
"""

import numpy as np
from contextlib import ExitStack

import concourse.bass as bass
import concourse.mybir as mybir
import concourse.tile as tile
from concourse import bacc
from concourse.bass_utils import run_bass_kernel_spmd

# Problem constants (hardcoded; kernel.py must be self-contained).
B = 4
LQ = 256
LK = 50000
OUT = 256
KV = 256            # input feature dim
NORM = 1.0 / 16.0   # 1/sqrt(OUT)

N_CORES = 8
SUB = 128                  # nodes per subtile (PE contraction width)
NSUB = 49                  # subtiles per core per batch
KS = NSUB * SUB            # 6272 nodes per core per batch
LK_PAD = KS * N_CORES      # 50176
GRP = 4                    # subtiles per exp/psum group (2 DoubleRow pairs,
NGRP = NSUB // GRP         # 2 PSUM banks) -> 12 groups; subtile 48 is odd
CHUNKS = (12, 12, 12, 8, 5)   # subtiles per DMA chunk (contiguous ranges)
NCHUNK = len(CHUNKS)
CH_OFF = tuple(sum(CHUNKS[:i]) for i in range(NCHUNK))
# Load order: the small tail chunk (groups g11 + odd subtile 48) comes
# FIRST so the odd group's serial exp chain overlaps the batch body
# instead of extending the batch tail.
CHUNK_ORDER = (4, 0, 1, 2, 3)
GROUP_ORDER = (12, 11, 0, 1, 2, 3, 4, 5, 6, 7, 8, 9, 10)  # 12 = odd group

SG = 64.0                        # score scale inside PSUM (folded into g)
A8 = 8 * np.log2(np.e) / SG      # Schraudolph fp8e4 slope
B8 = 56.05                       # 8*7 bias, +0.5 trunc->round, -0.45 mean-cal

# exp engine per group index: ScalarE / VectorE (Schraudolph), alternating
# in STREAM order (GROUP_ORDER) so the two exp engines run concurrently.
ACT_GROUPS = frozenset((12, 0, 2, 4, 6, 8, 10))   # odd + evens; g11 on DVE
POOL_GROUPS = frozenset()
ODD_ON_ACT = True  # kept for the __main__ reference model
PEND = 3                   # H matmuls lag their group by 3 so the PE FIFO
                           # never blocks upcoming scores on a pending exp

F8 = mybir.dt.float8e4
F16 = mybir.dt.float16
F32 = mybir.dt.float32
I8 = mybir.dt.int8
NPF8 = mybir.dt.np(mybir.dt.float8e4)
DR = mybir.MatmulPerfMode.DoubleRow


def _chunk_of(s):
    """DMA chunk index and subtile offset within the chunk."""
    for c in range(NCHUNK - 1, -1, -1):
        if s >= CH_OFF[c]:
            return c, s - CH_OFF[c]
    raise AssertionError(s)


def build(mode="full", pend_depth=PEND, flush_per_batch=False,
          g_first=False, merge_out=True, xt_lookahead=0):
    """Emit the per-core SPMD Bass module (identical on all cores).

    mode: "full" | "noh" (skip H matmuls+drain) | "noglo" (single G matmul)
    | "noexp" (static e tile; tests DMA+PE pipeline) — ablations for timing.
    """
    nc = bacc.Bacc("TRN2", target_bir_lowering=False, debug=False,
                   num_devices=N_CORES)
    xn = nc.dram_tensor("xn", [B, 128, NSUB, 257], F8, kind="ExternalInput")
    xt = nc.dram_tensor("xt", [B, 128, 2, KS], F8, kind="ExternalInput")
    # g[..., 0, :] = fp8(G), g[..., 1, :] = fp8(G - fp8(G)): the second,
    # accumulating matmul restores ~11-bit G precision. (A first-order host
    # correction does NOT work: the fp8 e-quantization's 12.5% buckets turn
    # the smooth score shift into quasi-random bucket flips.)
    g = nc.dram_tensor("g", [128, B, 2, 2, 256], F8, kind="ExternalInput")
    ht = nc.dram_tensor("ht", [B, 128, 2, 257], F16, kind="ExternalOutput")

    with ExitStack() as ctx:
        tc = ctx.enter_context(tile.TileContext(nc))
        gp = ctx.enter_context(tc.tile_pool(name="gp", bufs=1))
        xnp = ctx.enter_context(tc.tile_pool(name="xnp", bufs=B * NCHUNK))
        xtp = ctx.enter_context(tc.tile_pool(name="xtp", bufs=B * NCHUNK))
        ep = ctx.enter_context(tc.tile_pool(name="ep", bufs=4))
        eip = ctx.enter_context(tc.tile_pool(name="eip", bufs=4))
        hop = ctx.enter_context(tc.tile_pool(name="hop", bufs=4))
        spp = ctx.enter_context(tc.tile_pool(name="spp", bufs=3, space="PSUM"))
        hpp = ctx.enter_context(tc.tile_pool(name="hpp", bufs=1, space="PSUM"))

        # G hi+lo, resident: [i_lo(128 part), b, i_hi, hl, q]; loaded per
        # batch so batch 0's scores aren't gated on the whole tensor.
        g_sb = gp.tile([128, B, 2, 2, 256], F8)

        # Chunked input loads on one ordered queue (sync/SP), interleaved so
        # batch 0's first chunks land first; xt precedes xn since scores
        # only need xt+g.
        xn_tiles = {}
        xt_tiles = {}
        cmax = max(CHUNKS)
        def load_xt(b, c):
            s0, ns = CH_OFF[c], CHUNKS[c]
            xtt = xtp.tile([128, 2, cmax * SUB], F8, tag="xt")
            nc.sync.dma_start(
                out=xtt[:, :, :ns * SUB],
                in_=xt[b, :, :, s0 * SUB:(s0 + ns) * SUB])
            xt_tiles[(b, c)] = xtt

        def load_xn(b, c):
            s0, ns = CH_OFF[c], CHUNKS[c]
            xnt = xnp.tile([128, cmax, 257], F8, tag="xn")
            nc.sync.dma_start(out=xnt[:, :ns, :], in_=xn[b, :, s0:s0 + ns, :])
            xn_tiles[(b, c)] = xnt

        for b in range(B):
            if g_first:
                nc.sync.dma_start(out=g_sb[:, b], in_=g[:, b])
            order = list(CHUNK_ORDER)
            la = xt_lookahead
            for ci, c in enumerate(order):
                load_xt(b, c)
                if not g_first and ci == 1:
                    nc.sync.dma_start(out=g_sb[:, b], in_=g[:, b])
                if ci >= la:
                    load_xn(b, order[ci - la])
            for ci in range(len(order) - la, len(order)):
                load_xn(b, order[ci])

        # groups: (subtile list, engine) per batch, in GROUP_ORDER.
        def eng_of(g_):
            if g_ in POOL_GROUPS:
                return "pool"
            return "act" if g_ in ACT_GROUPS else "dve"
        def subs_of(g_):
            if g_ == NGRP:  # odd tail group
                return [NSUB - 1]
            return [g_ * GRP + i for i in range(GRP)]
        groups = [(subs_of(g_), eng_of(g_)) for g_ in GROUP_ORDER]
        # ONE global stream of (batch, group) — no per-batch pend flush, so
        # batch b+1's scores interleave with batch b's trailing H matmuls
        # and the exp engines never drain at batch boundaries.
        stream = [(b, sub, eng) for b in range(B) for sub, eng in groups]

        ht_acc = {}       # b -> (ht0, ht1)
        pend = []         # (b, e_f8, subs) whose H matmuls are not yet issued
        popped = [0] * B  # H groups issued per batch

        e_static = None
        if mode == "noexp":
            stp = ctx.enter_context(tc.tile_pool(name="stp", bufs=1))
            e_static = stp.tile([128, GRP, 256], F8, tag="es")
            nc.vector.memset(e_static[:, :, :], 1.0)

        def pop_h():
            b, pe_, psubs = pend.pop(0)
            if b not in ht_acc:
                ht_acc[b] = (hpp.tile([128, 257], F32, tag="ht0", name="ht0"),
                             hpp.tile([128, 257], F32, tag="ht1", name="ht1"))
            ht0, ht1 = ht_acc[b]
            first_grp = popped[b] == 0
            popped[b] += 1
            is_last_grp = popped[b] == len(groups)
            npair = len(psubs) // 2
            for i in range(npair):
                s = psubs[2 * i]
                c, off = _chunk_of(s)
                xnt = xn_tiles[(b, c)]
                first = first_grp and i == 0
                last = (is_last_grp and i == npair - 1
                        and len(psubs) % 2 == 0)
                for h, hacc in ((0, ht0), (1, ht1)):
                    nc.tensor.matmul(
                        hacc[:, :],
                        pe_[:, 2 * i:2 * i + 2, h * 128:(h + 1) * 128],
                        xnt[:, off:off + 2, 0:257],
                        start=first, stop=last, perf_mode=DR)
            if len(psubs) % 2:  # odd tail: plain fp8 matmul
                s = psubs[-1]
                c, off = _chunk_of(s)
                xnt = xn_tiles[(b, c)]
                i = len(psubs) - 1
                for h, hacc in ((0, ht0), (1, ht1)):
                    nc.tensor.matmul(
                        hacc[:, :],
                        pe_[:, i, h * 128:(h + 1) * 128],
                        xnt[:, off, 0:257],
                        start=first_grp and npair == 0, stop=is_last_grp)
            if is_last_grp:  # drain on both engines in parallel, out via SP
                hts = hop.tile([128, 2, 257], F16)
                nc.scalar.copy(hts[:, 0, :], ht0[:, :])
                nc.vector.tensor_copy(hts[:, 1, :], ht1[:, :])
                if merge_out:
                    nc.sync.dma_start(out=ht[b], in_=hts[:, :, :])
                else:
                    nc.sync.dma_start(out=ht[b, :, 0], in_=hts[:, 0, :])
                    nc.sync.dma_start(out=ht[b, :, 1], in_=hts[:, 1, :])

        for b, subs, eng in stream:
            sz = len(subs)
            sp = spp.tile([128, GRP, 256], F32)
            nglo = 1 if mode == "noglo" else 2
            for i in range(sz):
                c, off = _chunk_of(subs[i])
                xtt = xt_tiles[(b, c)]
                for hl in range(nglo):  # G hi then lo residual
                    nc.tensor.matmul(
                        sp[:, i, :],
                        xtt[:, :, off * SUB:(off + 1) * SUB],
                        g_sb[:, b, :, hl, :],
                        start=hl == 0, stop=hl == nglo - 1, perf_mode=DR)
            if mode == "noexp":
                e_f8 = e_static
            elif eng == "act":
                e = ep.tile([128, GRP, 256], F8, tag="ea")
                nc.scalar.activation(
                    e[:, :sz, :], sp[:, :sz, :],
                    mybir.ActivationFunctionType.Exp, scale=1.0 / SG)
                e_f8 = e
            else:
                ei = eip.tile([128, GRP, 256], I8, tag="ei")
                veng = nc.vector if eng == "dve" else nc.gpsimd
                veng.tensor_scalar(
                    ei[:, :sz, :], sp[:, :sz, :], A8, B8,
                    mybir.AluOpType.mult, mybir.AluOpType.add)
                e_f8 = ei[:, :, :].bitcast(F8)
            if mode != "noh":
                pend.append((b, e_f8, subs))
                # H matmuls lag pend_depth groups behind so the PE FIFO
                # never blocks upcoming scores on a pending exp.
                if len(pend) > pend_depth:
                    pop_h()
                if flush_per_batch and subs is groups[-1][0]:
                    while pend:
                        pop_h()
        while pend:
            pop_h()
    nc.compile()
    return nc


def _prepare_inputs(query, input, Wq, bq, Wk):
    """Host marshalling: folded G + fp8 input in both layouts, k-sharded.

    """
    # G[b] = Wk.T @ (query_b @ Wq.T + bq).T -> [B, 256(i), 256(q)], then
    # * NORM (1/16) * SG (64) so PSUM scores arrive pre-scaled by SG.
    Q = query.astype(np.float64) @ Wq.T.astype(np.float64) + bq
    G = np.einsum('di,bqd->biq', Wk.astype(np.float64), Q) * (NORM * SG)
    g_hi = G.astype(np.float32).astype(NPF8)
    g_lo = (G - g_hi.astype(np.float64)).astype(np.float32).astype(NPF8)
    # [i_lo, b, i_hi, hl, q] with i = i_hi*128 + i_lo
    g8 = np.ascontiguousarray(
        np.stack([g_hi, g_lo], axis=2)          # [B, 256, 2, 256]
        .reshape(B, 2, 128, 2, 256).transpose(2, 0, 1, 3, 4))

    xpad = np.zeros((B, LK_PAD, 257), np.float32)
    xpad[:, :LK, :256] = input
    xpad[:, :LK, 256] = 1.0   # ones-column -> denom; 0 on padded rows
    x8 = xpad.astype(NPF8)    # [B, LK_PAD, 258]

    in_maps = []
    for cid in range(N_CORES):
        sl = x8[:, cid * KS:(cid + 1) * KS, :]
        # natural: [B, 128(part), NSUB, 258]; node k = t*128 + p
        xn_c = sl.reshape(B, NSUB, 128, 257).transpose(0, 2, 1, 3)
        # transposed DoubleRow: [B, 128(i_lo), 2(i_hi), KS]
        xt_c = np.ascontiguousarray(
            sl[:, :, :256].transpose(0, 2, 1)).reshape(B, 2, 128, KS)
        xt_c = xt_c.transpose(0, 2, 1, 3)
        in_maps.append({
            "xn": np.ascontiguousarray(xn_c),
            "xt": np.ascontiguousarray(xt_c),
            "g": g8,
        })
    return in_maps


def kernel(query, input, Wq, bq, Wk, bk, Wv, bv):
    # bk provably cancels in softmax over k; bq folds into G; bv is applied
    # in the host epilogue.
    query = np.asarray(query, dtype=np.float32)
    input = np.asarray(input, dtype=np.float32)
    Wq = np.asarray(Wq, dtype=np.float32)
    bq = np.asarray(bq, dtype=np.float32)
    Wk = np.asarray(Wk, dtype=np.float32)
    Wv = np.asarray(Wv, dtype=np.float32)
    bv = np.asarray(bv, dtype=np.float32)

    nc = build()
    in_maps = _prepare_inputs(query, input, Wq, bq, Wk)
    res = run_bass_kernel_spmd(nc, in_maps, core_ids=list(range(N_CORES)))
    kernel._last_result = res

    numer = np.zeros((B, OUT))
    denom = np.zeros((B, OUT))
    Wv64 = Wv.astype(np.float64)
    for r in res.results:
        # [B, 128(p), 2(h), 257] -> [B, 256(j), 257] with j = h*128 + p
        H = (r["ht"].astype(np.float64)
             .transpose(0, 2, 1, 3).reshape(B, OUT, 257))
        numer += (H[:, :, :256] * Wv64[None]).sum(axis=2)
        denom += H[:, :, 256]
    out = numer / denom + bv
    return out.astype(np.float32)


if __name__ == "__main__":
    # CoreSim smoke test on the full module with random fp8 inputs.
    from concourse.bass_interp import CoreSim

    rng = np.random.default_rng(0)
    xpad = np.zeros((B, KS, 257), np.float32)
    xpad[:, :, :256] = rng.standard_normal((B, KS, 256))
    xpad[:, :, 256] = 1.0
    x8 = xpad.astype(NPF8)
    xn_np = np.ascontiguousarray(
        x8.reshape(B, NSUB, 128, 257).transpose(0, 2, 1, 3))
    xt_np = np.ascontiguousarray(
        x8[:, :, :256].transpose(0, 2, 1)).reshape(B, 2, 128, KS)
    xt_np = np.ascontiguousarray(xt_np.transpose(0, 2, 1, 3))
    g_np = (rng.standard_normal((B, 256, 256)) * 1.8).astype(np.float64)
    g_hi = g_np.astype(np.float32).astype(NPF8)
    g_lo = (g_np - g_hi.astype(np.float64)).astype(np.float32).astype(NPF8)
    g8 = np.ascontiguousarray(
        np.stack([g_hi, g_lo], axis=2)
        .reshape(B, 2, 128, 2, 256).transpose(2, 0, 1, 3, 4))

    nc = build()
    sim = CoreSim(nc)
    sim.tensor("xn")[:] = xn_np
    sim.tensor("xt")[:] = xt_np
    sim.tensor("g")[:] = g8
    sim.simulate()
    got = (np.array(sim.tensor("ht")).astype(np.float64)
           .transpose(0, 2, 1, 3).reshape(B, OUT, 257))

    x32 = x8.astype(np.float32)
    gsum = (g8[:, :, :, 0, :].astype(np.float32)
            + g8[:, :, :, 1, :].astype(np.float32))  # [il, b, ih, q]
    want = np.zeros((B, OUT, 257))
    for b in range(B):
        gb = gsum[:, b, :, :].transpose(1, 0, 2).reshape(256, 256)
        s = x32[b, :, :256] @ gb  # = SG * scores
        # per-subtile engine assignment
        e = np.zeros((KS, 256), np.float32)
        for t in range(NSUB):
            grp_i = t // GRP if t < NGRP * GRP else NGRP
            rows = slice(t * 128, (t + 1) * 128)
            # note: node k = t*128+p lives at partition p, subtile t; scores
            # rows here are k-major which matches
            if grp_i in ACT_GROUPS:
                e[rows] = np.exp(s[rows] / SG).astype(NPF8).astype(np.float32)
            else:
                y = np.trunc(A8 * s[rows] + B8).astype(np.int8)
                e[rows] = y.view(NPF8).astype(np.float32)
        want[b] = e.T @ x32[b, :, :257]
    err = np.abs(got - want).max() / np.abs(want).max()
    print("CoreSim rel err vs bit-exact model:", err)
    assert err < 1e-3, err
    print("OK")
